# revision 42
# baseline (speedup 1.0000x reference)
"""MoE (noisy top-2-of-8 gating) Trainium2 kernel.

Strategy: data-parallel over tokens (1024/core on 8 cores). The host computes
routing structure only (which expert each token goes to — the sharding
metadata); all FLOPs (gating values, expert MLPs, combine) run on device.

Tokens are dealt to cores round-robin within each (expert,expert) routing
class so every core sees a near-identical per-expert distribution — the SPMD
per-segment capacity (max over cores) then sits close to the global mean.
Per core the tokens are permuted into 8 expert segments (globally ordered by
descending count). All inputs are host-packed into exact SBUF layout so every
load is one large contiguous DMA (per-DMA fixed costs dominate small
transfers). Expert segments are processed in groups of two: fc1 (+exact-erf
GELU, batched two hidden-chunks per activation op) into hidden-major h tiles,
then fc2 with h-stationary matmuls, exp straight out of PSUM, one store per
segment into a padded-row DRAM exp-table, and the top-2 combine via
indirect-DMA row gathers + per-partition gate scaling + Ln.
"""

import numpy as np
import ml_dtypes

import concourse.bacc as bacc
import concourse.bass as bass
import concourse.mybir as mybir
import concourse.tile as tile
from concourse.bass_utils import run_bass_kernel_spmd
from concourse.masks import make_identity

BF16 = mybir.dt.bfloat16
FP32 = mybir.dt.float32
AF = mybir.ActivationFunctionType

N, D, H, E, TOPK = 8192, 512, 2048, 8, 2
NC = 8
NS = N // NC          # tokens per core
P = 128
NTT = NS // P         # token tiles per core (8)
DC = D // P           # d chunks (4)
HC = H // P           # hidden chunks (16)
FC = (2 * D) // P     # gate feature chunks (8)
W1C = DC * H          # w1 SBUF cols (8192)
W2C = HC * D          # w2 SBUF cols (8192)

_nc_cache: dict = {}


def _build_nc(caps, rsegs=(7,) * 8, reps=1, timing=False, use_b1=False,
              use_b2=False, wbufs=3, hbufs=3, abufs=2, QS=2, skip=()):
    """Build the SPMD Bass program for per-segment capacities `caps`.

    timing=True makes all data tensors internal DRAM (no host transfer) and
    the output a dummy, so repeated-execution wall-clock isolates device time.
    """
    caps = tuple(int(c) for c in caps)
    R = sum(caps)
    offs = np.concatenate([[0], np.cumsum(caps)]).astype(int)
    ntts = [(c + P - 1) // P for c in caps]
    offp = np.concatenate([[0], np.cumsum([n * P for n in ntts])]).astype(int)
    R_pad = int(offp[-1])

    nc = bacc.Bacc("TRN2", target_bir_lowering=False, debug=False)

    if timing:
        def param(name, shape, dtype):
            return nc.dram_tensor(name, shape, dtype)
        nc.declare_dram_parameter("tdin", [1, 4], FP32, isOutput=False)
        y_d = nc.dram_tensor("y", [NS, D], BF16)
        yo_d = nc.declare_dram_parameter("yo", [1, 4], FP32, isOutput=True)
    else:
        def param(name, shape, dtype):
            return nc.declare_dram_parameter(name, shape, dtype, isOutput=False)
        y_d = nc.declare_dram_parameter("y", [NS, D], BF16, isOutput=True)

    xt_d = param("xt", [P, DC * R], BF16)
    gft_d = param("gft", [P, FC * NS], BF16)
    nst_d = param("nst", [E, NS], FP32)
    wgwn_d = param("wgwn", [P, FC * 2 * E], BF16)
    w_d = param("w", [E, P, W1C + W2C], BF16)
    j12_d = param("j12", [P, 2 * NTT], mybir.dt.int32)
    sm_d = param("sm", [P, 8], FP32)
    if use_b1:
        b1_d = param("b1", [P, E * HC], FP32)
    if use_b2:
        b2_d = param("b2", [E, D], BF16)

    with tile.TileContext(nc) as tc:
        with (
            tc.tile_pool(name="const", bufs=1) as constp,
            tc.tile_pool(name="gate", bufs=1) as gatep,
            tc.tile_pool(name="wpool", bufs=wbufs) as wp,
            tc.tile_pool(name="hpool", bufs=hbufs) as hp,
            tc.tile_pool(name="apool", bufs=abufs) as ap_,
            tc.tile_pool(name="spool", bufs=2) as sp,
            tc.tile_pool(name="psum1", bufs=2, space="PSUM") as pp,
            tc.tile_pool(name="psum2", bufs=2, space="PSUM") as pp2,
            tc.tile_pool(name="psumg", bufs=2, space="PSUM") as ppg,
            tc.tile_pool(name="dram", bufs=1, space="DRAM") as dp,
        ):
            ident = constp.tile([P, P], FP32)
            make_identity(nc, ident[:])
            if use_b2:
                ones1 = constp.tile([1, P], BF16)
                nc.vector.memset(ones1[:], 1.0)

            def body(_i=None):
                tiles_last = [t for t in range(NTT) if rsegs[t] == E - 1]
                a_dram = dp.tile([R_pad, D], BF16, tag="a_tab")

                # ---------- input loads (few, large, pre-swizzled) ----------
                # x columns of segment 0 first so fc1 can start immediately
                xsb = gatep.tile([P, DC * R], BF16, tag="xsb")
                c0 = caps[0]
                nc.sync.dma_start(
                    out=xsb[:].rearrange("p (d r) -> p d r", r=R)[:, :, 0:c0],
                    in_=xt_d[:].rearrange("p (d r) -> p d r", r=R)[:, :, 0:c0],
                )
                # segment-0 fc1 weights in quarter slices so matmuls start early
                wsb = {}
                wsb[0] = wp.tile([P, W1C + W2C], BF16, tag="w", name="wsb0")
                for q in range(4):
                    nc.sync.dma_start(
                        out=wsb[0][:, q * (W1C // 4) : (q + 1) * (W1C // 4)],
                        in_=w_d[0][:, q * (W1C // 4) : (q + 1) * (W1C // 4)],
                    )
                gfsb = gatep.tile([P, FC * NS], BF16, tag="gfsb")
                nc.sync.dma_start(out=gfsb[:], in_=gft_d[:])
                wgwn = gatep.tile([P, FC * 2 * E], BF16, tag="wgwn")
                nc.sync.dma_start(out=wgwn[:], in_=wgwn_d[:])
                # rest of x
                nc.sync.dma_start(
                    out=xsb[:].rearrange("p (d r) -> p d r", r=R)[:, :, c0:R],
                    in_=xt_d[:].rearrange("p (d r) -> p d r", r=R)[:, :, c0:R],
                )
                wsb[1] = wp.tile([P, W1C + W2C], BF16, tag="w", name="wsb1")
                nc.sync.dma_start(out=wsb[1][:, :W1C], in_=w_d[1][:, :W1C])
                nc.sync.dma_start(out=wsb[0][:, W1C:], in_=w_d[0][:, W1C:])
                nc.sync.dma_start(out=wsb[1][:, W1C:], in_=w_d[1][:, W1C:])
                nssb = gatep.tile([E, NS], FP32, tag="nssb")
                nc.sync.dma_start(out=nssb[:], in_=nst_d[:])
                j12sb = gatep.tile([P, 2 * NTT], mybir.dt.int32, tag="j12sb")
                smsb = gatep.tile([P, 8], FP32, tag="smsb")
                if timing:
                    # internal j tensor holds garbage; keep gather rows at 0
                    nc.vector.memset(j12sb[:], 0)
                    nc.vector.memset(smsb[:], 1.0)
                else:
                    nc.sync.dma_start(out=j12sb[:], in_=j12_d[:])
                    nc.sync.dma_start(out=smsb[:], in_=sm_d[:])
                if use_b1:
                    b1sb = gatep.tile([P, E * HC], FP32, tag="b1sb")
                    nc.sync.dma_start(out=b1sb[:], in_=b1_d[:])
                if use_b2:
                    b2sb = gatep.tile([E, D], BF16, tag="b2sb")
                    nc.sync.dma_start(out=b2sb[:], in_=b2_d[:])

                hsb = {}
                ssb = gatep.tile([P, NTT * D], BF16, tag="ssb")
                lg_sb = gatep.tile([E, NS], FP32, tag="lg")
                lgc_sb = gatep.tile([E, NS], FP32, tag="lgc")
                lgn_sb = gatep.tile([E, NS], FP32, tag="lgn")
                g1sb = gatep.tile([P, NTT], FP32, tag="g1")
                g2sb = gatep.tile([P, NTT], FP32, tag="g2")
                gab = gatep.tile([P, 4 * max(1, len(tiles_last))], FP32,
                                 tag="gab")
                if "gate" in skip:
                    nc.vector.memset(g1sb[:], 0.5)
                    nc.vector.memset(g2sb[:], 0.5)
                    nc.vector.memset(gab[:], 0.5)

                def emit_fc1(k):
                    cap = caps[k]
                    off = int(offs[k])
                    hsb[k] = hp.tile([P, HC * cap], BF16, tag="h", name=f"hsb{k}")
                    for hp2 in range(HC // 2 if "fc1" not in skip else 1):
                        ps = pp.tile([P, 1024], FP32, tag="fc1_ps")
                        for h2 in range(2):
                            h = hp2 * 2 + h2
                            for d in range(DC):
                                nc.tensor.matmul(
                                    ps[:, h2 * 512 : h2 * 512 + cap],
                                    lhsT=wsb[k][:, h * 512 + d * P : h * 512 + (d + 1) * P],
                                    rhs=xsb[:, d * R + off : d * R + off + cap],
                                    start=(d == 0),
                                    stop=(d == DC - 1),
                                )
                        if use_b1:
                            for h2 in range(2):
                                h = hp2 * 2 + h2
                                nc.scalar.activation(
                                    hsb[k][:, h * cap : (h + 1) * cap],
                                    ps[:, h2 * 512 : h2 * 512 + cap],
                                    AF.Gelu,
                                    bias=b1sb[:, k * HC + h : k * HC + h + 1],
                                )
                        else:
                            nc.scalar.activation(
                                hsb[k][:].rearrange("p (h n) -> p h n", n=cap)[
                                    :, hp2 * 2 : hp2 * 2 + 2, :
                                ],
                                ps[:].rearrange("p (h n) -> p h n", n=512)[
                                    :, :, 0:cap
                                ],
                                AF.Gelu,
                            )

                def emit_gating_mm():
                    # clean and noise logits into SBUF via small psum chunks
                    for t in range(2):
                        t0 = t * 512
                        for j, dst in ((0, lgc_sb), (E, lgn_sb)):
                            gps = ppg.tile([E, 512], FP32, tag="gate_ps")
                            for c in range(FC):
                                nc.tensor.matmul(
                                    gps[:],
                                    lhsT=wgwn[:, c * 2 * E + j : c * 2 * E + j + E],
                                    rhs=gfsb[:, c * NS + t0 : c * NS + t0 + 512],
                                    start=(c == 0),
                                    stop=(c == FC - 1),
                                )
                            nc.vector.tensor_copy(dst[:, t0 : t0 + 512], gps[:])

                def emit_gate_softplus():
                    # stddev = softplus(noise_logits) + 1e-2; logits = clean + noise*std
                    nc.scalar.activation(lgn_sb[:], lgn_sb[:], AF.Exp)
                    nc.vector.tensor_scalar_add(lgn_sb[:], lgn_sb[:], 1.0)
                    nc.scalar.activation(lgn_sb[:], lgn_sb[:], AF.Ln)
                    nc.vector.tensor_scalar_add(lgn_sb[:], lgn_sb[:], 1e-2)
                    nc.vector.tensor_mul(lgn_sb[:], lgn_sb[:], nssb[:])
                    nc.vector.tensor_add(lg_sb[:], lgn_sb[:], lgc_sb[:])

                def emit_gating_post():
                    # transpose to [tok, E] tiles; top-2 softmax via sigmoid
                    trp = ppg.tile([P, NTT * E], FP32, tag="gate_ps")
                    for t in range(NTT):
                        nc.tensor.transpose(
                            trp[:, t * E : (t + 1) * E],
                            lg_sb[:, t * P : (t + 1) * P],
                            ident[:E, :E],
                        )
                    lt8 = sp.tile([P, NTT * E], FP32, tag="lt8")
                    nc.vector.tensor_copy(lt8[:], trp[:])
                    mx8 = sp.tile([P, NTT * 8], FP32, tag="mx8")
                    d21 = sp.tile([P, NTT], FP32, tag="d21")
                    e21 = sp.tile([P, NTT], FP32, tag="e21")
                    t1g = sp.tile([P, NTT], FP32, tag="t1g")
                    for t in range(NTT):
                        nc.vector.max(
                            out=mx8[:, t * 8 : (t + 1) * 8],
                            in_=lt8[:, t * E : (t + 1) * E],
                        )
                        nc.vector.tensor_sub(
                            d21[:, t : t + 1],
                            mx8[:, t * 8 + 1 : t * 8 + 2],
                            mx8[:, t * 8 : t * 8 + 1],
                        )
                    # g1 = sigmoid(v1-v2) = 1/(1+e), g2 = 1-g1 = g1*e
                    nc.scalar.activation(e21[:], d21[:], AF.Exp)
                    nc.vector.tensor_scalar_add(t1g[:], e21[:], 1.0)
                    nc.vector.reciprocal(g1sb[:], t1g[:])
                    nc.vector.tensor_mul(g2sb[:], g1sb[:], e21[:])

                def emit_fc2(k):
                    cap = caps[k]
                    ntt = ntts[k]
                    aout = ap_.tile([P, ntt * D], BF16, tag="aout")
                    for tt in range(ntt):
                        m = min(P, cap - tt * P)
                        ps2 = pp2.tile([P, D], FP32, tag="fc2_ps")
                        for h in range(HC if "fc2" not in skip else 1):
                            nc.tensor.matmul(
                                ps2[:m],
                                lhsT=hsb[k][:, h * cap + tt * P : h * cap + tt * P + m],
                                rhs=wsb[k][:, W1C + h * D : W1C + (h + 1) * D],
                                start=(h == 0),
                                stop=(h == (HC if "fc2" not in skip else 1) - 1
                                      and not use_b2),
                            )
                        if use_b2:
                            nc.tensor.matmul(
                                ps2[:m],
                                lhsT=ones1[:, :m],
                                rhs=b2sb[k : k + 1, :],
                                start=False,
                                stop=True,
                            )
                        # full-tile exp (pad rows hold stale-but-finite psum)
                        nc.scalar.activation(
                            aout[:, tt * D : (tt + 1) * D], ps2[:], AF.Exp
                        )
                    del hsb[k]
                    if k == E - 1:
                        # last segment's rows are pinned per final-tile token
                        # slot; combines read them straight from SBUF
                        aout_last.append(aout)
                    elif "astore" not in skip:
                        nc.sync.dma_start(
                            out=a_dram[offp[k] : offp[k + 1]].rearrange(
                                "(t p) j -> p t j", p=P
                            ),
                            in_=aout[:].rearrange("p (t j) -> p t j", j=D),
                        )

                def emit_combine(t):
                    # gate-weighted sum parked into ssb; Ln + y output happen
                    # once at the end so ACT never blocks the gelu pipeline.
                    # both expert rows come from one paired indirect gather.
                    if rsegs[t] == E - 1:
                        tt0 = t - (NTT - ntts[E - 1])
                        pref = int(offp[E - 1]) + (tt0 + 1) * P
                    else:
                        pref = int(offp[rsegs[t] + 1])
                    b12 = sp.tile([P, 2 * D], BF16, tag="b12")
                    nc.gpsimd.indirect_dma_start(
                        out=b12[:, :D],
                        out_offset=None,
                        in_=a_dram[0:pref, :],
                        in_offset=bass.IndirectOffsetOnAxis(
                            ap=j12sb[:, 2 * t : 2 * t + 1], axis=0
                        ),
                    )
                    nc.gpsimd.indirect_dma_start(
                        out=b12[:, D:],
                        out_offset=None,
                        in_=a_dram[0:pref, :],
                        in_offset=bass.IndirectOffsetOnAxis(
                            ap=j12sb[:, 2 * t + 1 : 2 * t + 2], axis=0
                        ),
                    )
                    s2 = sp.tile([P, D], FP32, tag="s2")
                    nc.vector.tensor_scalar_mul(s2[:], b12[:, D:], g2sb[:, t : t + 1])
                    nc.vector.scalar_tensor_tensor(
                        ssb[:, t * D : (t + 1) * D],
                        b12[:, :D],
                        g1sb[:, t : t + 1],
                        s2[:],
                        mybir.AluOpType.mult,
                        mybir.AluOpType.add,
                    )

                def emit_gab():
                    # gA = g1 + s*(g2-g1), gB = (g1+g2) - gA for the last tiles
                    for i, t in enumerate(tiles_last):
                        tt0 = t - (NTT - ntts[E - 1])
                        nc.vector.tensor_sub(gab[:, 4 * i + 2 : 4 * i + 3],
                                             g2sb[:, t : t + 1],
                                             g1sb[:, t : t + 1])
                        nc.vector.tensor_mul(gab[:, 4 * i + 2 : 4 * i + 3],
                                             gab[:, 4 * i + 2 : 4 * i + 3],
                                             smsb[:, 4 * tt0 : 4 * tt0 + 1])
                        nc.vector.tensor_add(gab[:, 4 * i : 4 * i + 1],
                                             g1sb[:, t : t + 1],
                                             gab[:, 4 * i + 2 : 4 * i + 3])
                        nc.vector.tensor_add(gab[:, 4 * i + 3 : 4 * i + 4],
                                             g1sb[:, t : t + 1],
                                             g2sb[:, t : t + 1])
                        nc.vector.tensor_sub(gab[:, 4 * i + 1 : 4 * i + 2],
                                             gab[:, 4 * i + 3 : 4 * i + 4],
                                             gab[:, 4 * i : 4 * i + 1])

                def emit_combine_last(t):
                    # side A: the non-last-segment row, gathered early (rows
                    # all sit below offp[E-1]); side B: the pinned last-segment
                    # row read directly from the aout SBUF tile, with a
                    # gathered fallback for tokens routed entirely below
                    tt0 = t - (NTT - ntts[E - 1])
                    direct = aout_last[0][:, tt0 * D : (tt0 + 1) * D]
                    pref = int(offp[E - 1])
                    bA = sp.tile([P, D], BF16, tag="bA")
                    nc.gpsimd.indirect_dma_start(
                        out=bA[:],
                        out_offset=None,
                        in_=a_dram[0:pref, :],
                        in_offset=bass.IndirectOffsetOnAxis(
                            ap=j12sb[:, 2 * t : 2 * t + 1], axis=0
                        ),
                    )
                    i = tiles_last.index(t)
                    gA = gab[:, 4 * i : 4 * i + 1]
                    gB = gab[:, 4 * i + 1 : 4 * i + 2]
                    s2 = sp.tile([P, D], FP32, tag="s2")
                    if t == tiles_last[0]:
                        # blend in the straggler fallback rows (tile 0 only)
                        bS = sp.tile([P, D], BF16, tag="bS")
                        nc.gpsimd.indirect_dma_start(
                            out=bS[:],
                            out_offset=None,
                            in_=a_dram[0:pref, :],
                            in_offset=bass.IndirectOffsetOnAxis(
                                ap=j12sb[:, 2 * t + 1 : 2 * t + 2], axis=0
                            ),
                        )
                        bB = sp.tile([P, D], FP32, tag="bB")
                        nc.vector.tensor_sub(bB[:], direct, bS[:])
                        nc.vector.tensor_scalar_mul(
                            bB[:], bB[:], smsb[:, 4 * tt0 + 1 : 4 * tt0 + 2]
                        )
                        nc.vector.tensor_add(bB[:], bB[:], bS[:])
                        nc.scalar.activation(s2[:], bB[:], AF.Copy, scale=gB)
                    else:
                        nc.scalar.activation(s2[:], direct, AF.Copy, scale=gB)
                    nc.vector.scalar_tensor_tensor(
                        ssb[:, t * D : (t + 1) * D],
                        bA[:],
                        gA,
                        s2[:],
                        mybir.AluOpType.mult,
                        mybir.AluOpType.add,
                    )

                def emit_ln_y(t0, t1):
                    # ln over parked tiles [t0, t1) and the matching y store
                    if t1 <= t0:
                        return
                    nc.scalar.activation(
                        ssb[:, t0 * D : t1 * D], ssb[:, t0 * D : t1 * D], AF.Ln
                    )
                    nc.sync.dma_start(
                        out=y_d[t0 * P : t1 * P].rearrange("(t p) j -> p t j", p=P),
                        in_=ssb[:, t0 * D : t1 * D].rearrange(
                            "p (t j) -> p t j", j=D
                        ),
                    )

                # early tiles are parked before the last group starts; their
                # Ln+y run in the last group's fc2 slack, rest at the end
                nsplit = sum(1 for t in range(NTT) if rsegs[t] <= E - 3)
                aout_last = []

                # ---------- main schedule: groups of QS segments ----------
                for g in range(E // QS):
                    k0, k1 = g * QS, g * QS + QS - 1
                    for k in range(k0, k1 + 1):
                        emit_fc1(k)
                        if k == 0 and "gate" not in skip:
                            emit_gating_mm()
                    if g == 0 and "gate" not in skip:
                        emit_gate_softplus()
                    # prefetch next group's weights: one whole-segment DMA
                    # each (prefetch slack far exceeds the transfer time)
                    for kn in range(k1 + 1, min(k1 + 1 + QS, E)):
                        wsb[kn] = wp.tile([P, W1C + W2C], BF16, tag="w",
                                          name=f"wsb{kn}")
                        if "wdma" in skip:
                            nc.sync.dma_start(out=wsb[kn][:, :128],
                                              in_=w_d[kn][:, :128])
                        else:
                            nc.sync.dma_start(out=wsb[kn][:], in_=w_d[kn][:])
                    emit_fc2(k0)
                    del wsb[k0]
                    for t in range(NTT):
                        if rsegs[t] == k0 and "combine" not in skip:
                            emit_combine(t)
                    if g == E // QS - 1:
                        emit_ln_y(0, nsplit)
                    emit_fc2(k1)
                    del wsb[k1]
                    for t in range(NTT):
                        if rsegs[t] == k1 and "combine" not in skip:
                            if k1 == E - 1:
                                emit_combine_last(t)
                            else:
                                emit_combine(t)
                    if g == 0 and "gate" not in skip:
                        emit_gating_post()
                        emit_gab()
                if "combine" not in skip:
                    nltail = min(tiles_last) if tiles_last else NTT
                    if nltail > nsplit:
                        emit_ln_y(nsplit, nltail)
                    emit_ln_y(nltail, NTT)

            if reps > 1:
                with tc.For_i(0, reps, 1):
                    body()
            else:
                body()
            if timing:
                nc.sync.dma_start(out=yo_d[:], in_=ident[:1, :4])

    nc.compile()
    return nc


def _route(gate_feat, noise, w_gate, w_noise):
    """Host-side routing structure (fp32 numpy, matches jax top-k selection)."""
    clean = gate_feat @ w_gate
    stddev = np.logaddexp(gate_feat @ w_noise, 0.0) + np.float32(1e-2)
    logits = clean.astype(np.float32) + noise * stddev.astype(np.float32)
    top2 = np.argsort(-logits, axis=1, kind="stable")[:, :TOPK].astype(np.int32)
    return top2


def _prepare(x, gate_feat, noise, w_gate, w_noise, fc1_w, fc1_b, fc2_w, fc2_b):
    x = np.asarray(x, dtype=np.float32)
    gate_feat = np.asarray(gate_feat, dtype=np.float32)
    noise = np.asarray(noise, dtype=np.float32)
    bf = ml_dtypes.bfloat16

    top2 = _route(gate_feat, noise, w_gate, w_noise)

    # deal tokens to cores round-robin within each expert-pair class so all
    # cores see a near-identical per-expert load
    pairs = np.sort(top2, axis=1)
    key = pairs[:, 0] * E + pairs[:, 1]
    sidx = np.argsort(key, kind="stable")
    toks = [sidx[c::NC] for c in range(NC)]

    cnt_global = np.bincount(top2.ravel(), minlength=E)
    order = np.argsort(-cnt_global, kind="stable").astype(np.int64)
    seg_of_expert = np.empty(E, dtype=np.int64)
    seg_of_expert[order] = np.arange(E)

    core_meta = []
    caps = np.zeros(E, dtype=np.int64)
    for c in range(NC):
        t2 = top2[toks[c]]
        pair_seg = seg_of_expert[t2.ravel()]
        caps = np.maximum(caps, np.bincount(pair_seg, minlength=E))
        core_meta.append(pair_seg)

    # pin the last (smallest) segment to exactly two fixed tiles: its rows sit
    # at slot (perm position - (NS - 2P)) so the final token tiles can read
    # them straight from the producing SBUF tile (no store+gather round trip)
    last_pin = bool(caps[E - 1] <= 2 * P) and all(
        np.sum(m == E - 1) >= P for m in core_meta
    )
    if last_pin:
        caps[E - 1] = 2 * P
    ntts = (caps + P - 1) // P
    offs = np.concatenate([[0], np.cumsum(caps)]).astype(np.int64)
    offp = np.concatenate([[0], np.cumsum(ntts * P)]).astype(np.int64)
    R = int(offs[-1])

    # ---- pass 1: per-core routing rows / readiness ----
    percore = []
    rsegs_cores = []
    for c in range(NC):
        pair_seg = core_meta[c]
        sort_idx = np.argsort(pair_seg, kind="stable")
        seg_counts = np.bincount(pair_seg, minlength=E)
        seg_starts = np.concatenate([[0], np.cumsum(seg_counts)])
        pos_in_seg = np.arange(2 * NS) - seg_starts[pair_seg[sort_idx]]
        rows_sorted = offp[pair_seg[sort_idx]] + pos_in_seg
        rows_of_pair = np.empty(2 * NS, dtype=np.int64)
        rows_of_pair[sort_idx] = rows_sorted

        # readiness: last segment a token's pair rows land in; sort tokens so
        # early-ready tokens combine while later segments still compute
        ready = np.maximum(pair_seg[0::2], pair_seg[1::2])
        perm = np.argsort(ready, kind="stable")
        rsegs_cores.append(ready[perm].reshape(NTT, P).max(axis=1))

        if last_pin:
            tokpos = np.empty(NS, dtype=np.int64)
            tokpos[perm] = np.arange(NS)
            lastmask = pair_seg == E - 1
            tp = tokpos[np.arange(2 * NS) // 2][lastmask]
            assert np.all(tp >= NS - 2 * P), "last-seg rows must sit in final tiles"
            rows_of_pair[lastmask] = offp[E - 1] + (tp - (NS - 2 * P))
        percore.append((pair_seg, sort_idx, seg_counts, seg_starts,
                        rows_of_pair, ready, perm))

    rsegs = tuple(int(v) for v in np.max(np.stack(rsegs_cores), axis=0))
    tiles_last = [t for t in range(NTT) if rsegs[t] == E - 1] if last_pin else []

    # ---- shared weight packing (global segment order) ----
    w1p = np.transpose(
        np.asarray(fc1_w).reshape(E, HC, P, DC, P), (0, 4, 1, 3, 2)
    ).reshape(E, P, W1C)
    w2p = np.transpose(
        np.asarray(fc2_w).reshape(E, D, HC, P), (0, 3, 2, 1)
    ).reshape(E, P, W2C)
    w_all = np.ascontiguousarray(
        np.concatenate([w1p, w2p], axis=2)[order]
    ).astype(bf)

    wg_r = np.asarray(w_gate, np.float32).reshape(FC, P, E)
    wn_r = np.asarray(w_noise, np.float32).reshape(FC, P, E)
    wgwn = np.ascontiguousarray(
        np.transpose(np.concatenate([wg_r, wn_r], axis=2), (1, 0, 2)).reshape(
            P, FC * 2 * E
        )
    ).astype(bf)

    use_b1 = bool(np.any(np.asarray(fc1_b)))
    use_b2 = bool(np.any(np.asarray(fc2_b)))
    b1p = np.ascontiguousarray(
        np.transpose(np.asarray(fc1_b, np.float32)[order].reshape(E, HC, P),
                     (2, 0, 1)).reshape(P, E * HC)
    )
    b2p = np.ascontiguousarray(np.asarray(fc2_b)[order]).astype(bf)

    # ---- pass 2: per-core input packing ----
    in_maps = []
    gidxs = []
    for c in range(NC):
        (pair_seg, sort_idx, seg_counts, seg_starts,
         rows_of_pair, ready, perm) = percore[c]
        j1 = rows_of_pair[0::2].astype(np.int32)
        j2 = rows_of_pair[1::2].astype(np.int32)

        # x columns in segment order, padded per segment (unpadded offsets)
        tok_sorted = sort_idx // 2
        cols = np.zeros(R, dtype=np.int64)
        for k in range(E):
            s0, n = int(seg_starts[k]), int(seg_counts[k])
            cols[offs[k] : offs[k] + n] = tok_sorted[s0 : s0 + n]
        if last_pin:
            cols[offs[E - 1] : offs[E - 1] + 2 * P] = perm[NS - 2 * P :]
        x_loc = x[toks[c]]
        xt = np.ascontiguousarray(
            np.transpose(x_loc[cols].reshape(R, DC, P), (2, 1, 0)).reshape(
                P, DC * R
            )
        ).astype(bf)

        gf = gate_feat[toks[c]][perm]
        gft = np.ascontiguousarray(
            np.transpose(gf.reshape(NS, FC, P), (2, 1, 0)).reshape(P, FC * NS)
        ).astype(bf)
        nst = np.ascontiguousarray(noise[toks[c]][perm].T.astype(np.float32))

        # j12 pairs adjacent: col 2t = top-1 row, 2t+1 = top-2 row of tile t.
        # tiles handled by combine_last get (jA = non-last-seg row, jB2 =
        # straggler fallback row) plus the s/m selectors in sm
        j1t = j1[perm].reshape(NTT, P)
        j2t = j2[perm].reshape(NTT, P)
        s71 = (pair_seg[0::2] == E - 1)[perm].reshape(NTT, P)
        s72 = (pair_seg[1::2] == E - 1)[perm].reshape(NTT, P)
        sm = np.zeros((P, 8), dtype=np.float32)
        for i, t in enumerate(tiles_last):
            has7 = s71[t] | s72[t]
            if t != tiles_last[0]:
                assert has7.all(), "stragglers must sit in the first last-tile"
            ja = np.where(s71[t], j2t[t], j1t[t])
            jb2 = np.where(has7, 0, j2t[t])
            j1t[t], j2t[t] = ja, jb2
            tt0 = t - (NTT - int(ntts[E - 1]))
            sm[:, 4 * tt0 + 0] = s71[t].astype(np.float32)
            sm[:, 4 * tt0 + 1] = has7.astype(np.float32)
        j12 = np.ascontiguousarray(
            np.stack([j1t.T, j2t.T], axis=2).reshape(P, 2 * NTT)
        )
        im = {
            "xt": xt,
            "gft": gft,
            "nst": nst,
            "wgwn": wgwn,
            "w": w_all,
            "j12": j12,
            "sm": sm,
        }
        if use_b1:
            im["b1"] = b1p
        if use_b2:
            im["b2"] = b2p
        in_maps.append(im)
        gidxs.append(toks[c][perm])

    return tuple(int(v) for v in caps), rsegs, gidxs, in_maps


def kernel(x, gate_feat, noise, w_gate, w_noise, fc1_w, fc1_b, fc2_w, fc2_b,
           _reps=1):
    caps, rsegs, gidxs, in_maps = _prepare(
        x, gate_feat, noise, w_gate, w_noise, fc1_w, fc1_b, fc2_w, fc2_b
    )
    use_b1 = "b1" in in_maps[0]
    use_b2 = "b2" in in_maps[0]
    key = (caps, rsegs, int(_reps), use_b1, use_b2)
    if key not in _nc_cache:
        _nc_cache[key] = _build_nc(caps, rsegs, reps=_reps, use_b1=use_b1,
                                   use_b2=use_b2)
    nc = _nc_cache[key]
    try:
        res = run_bass_kernel_spmd(nc, in_maps, core_ids=list(range(NC)))
    except Exception:
        # transient device wedge (seen once as NRT_EXEC_UNIT_UNRECOVERABLE on a
        # cold device); one retry after the runtime recovers
        res = run_bass_kernel_spmd(nc, in_maps, core_ids=list(range(NC)))
    y = np.empty((N, D), np.float32)
    for c in range(NC):
        y[gidxs[c]] = res.results[c]["y"]
    return y


# revision 44
# speedup vs baseline: 1.0195x; 1.0195x over previous
"""MoE (noisy top-2-of-8 gating) Trainium2 kernel.

Strategy: data-parallel over tokens (1024/core on 8 cores). The host computes
routing structure only (which expert each token goes to — the sharding
metadata); all FLOPs (gating values, expert MLPs, combine) run on device.

Tokens are dealt to cores round-robin within each (expert,expert) routing
class so every core sees a near-identical per-expert distribution — the SPMD
per-segment capacity (max over cores) then sits close to the global mean.
Per core the tokens are permuted into 8 expert segments (globally ordered by
descending count). All inputs are host-packed into exact SBUF layout so every
load is one large contiguous DMA (per-DMA fixed costs dominate small
transfers). Expert segments are processed in groups of two: fc1 (+exact-erf
GELU, batched two hidden-chunks per activation op) into hidden-major h tiles,
then fc2 with h-stationary matmuls, exp straight out of PSUM, one store per
segment into a padded-row DRAM exp-table, and the top-2 combine via
indirect-DMA row gathers + per-partition gate scaling + Ln.
"""

import numpy as np
import ml_dtypes

import concourse.bacc as bacc
import concourse.bass as bass
import concourse.mybir as mybir
import concourse.tile as tile
from concourse.bass_utils import run_bass_kernel_spmd
from concourse.masks import make_identity

BF16 = mybir.dt.bfloat16
FP32 = mybir.dt.float32
AF = mybir.ActivationFunctionType

N, D, H, E, TOPK = 8192, 512, 2048, 8, 2
NC = 8
NS = N // NC          # tokens per core
P = 128
NTT = NS // P         # token tiles per core (8)
DC = D // P           # d chunks (4)
HC = H // P           # hidden chunks (16)
FC = (2 * D) // P     # gate feature chunks (8)
W1C = DC * H          # w1 SBUF cols (8192)
W2C = HC * D          # w2 SBUF cols (8192)

_nc_cache: dict = {}


def _build_nc(caps, rsegs=(7,) * 8, reps=1, timing=False, use_b1=False,
              use_b2=False, wbufs=3, hbufs=3, abufs=3, QS=2, skip=()):
    """Build the SPMD Bass program for per-segment capacities `caps`.

    timing=True makes all data tensors internal DRAM (no host transfer) and
    the output a dummy, so repeated-execution wall-clock isolates device time.
    """
    caps = tuple(int(c) for c in caps)
    R = sum(caps)
    offs = np.concatenate([[0], np.cumsum(caps)]).astype(int)
    ntts = [(c + P - 1) // P for c in caps]
    offp = np.concatenate([[0], np.cumsum([n * P for n in ntts])]).astype(int)
    R_pad = int(offp[-1])

    nc = bacc.Bacc("TRN2", target_bir_lowering=False, debug=False)

    if timing:
        def param(name, shape, dtype):
            return nc.dram_tensor(name, shape, dtype)
        nc.declare_dram_parameter("tdin", [1, 4], FP32, isOutput=False)
        y_d = nc.dram_tensor("y", [NS, D], BF16)
        yo_d = nc.declare_dram_parameter("yo", [1, 4], FP32, isOutput=True)
    else:
        def param(name, shape, dtype):
            return nc.declare_dram_parameter(name, shape, dtype, isOutput=False)
        y_d = nc.declare_dram_parameter("y", [NS, D], BF16, isOutput=True)

    xt_d = param("xt", [P, DC * R], BF16)
    gft_d = param("gft", [P, FC * NS], BF16)
    nst_d = param("nst", [E, NS], FP32)
    wgwn_d = param("wgwn", [P, FC * 2 * E], BF16)
    w_d = param("w", [E, P, W1C + W2C], BF16)
    j12_d = param("j12", [P, 2 * NTT], mybir.dt.int32)
    sm_d = param("sm", [P, 8], FP32)
    if use_b1:
        b1_d = param("b1", [P, E * HC], FP32)
    if use_b2:
        b2_d = param("b2", [E, D], BF16)

    with tile.TileContext(nc) as tc:
        with (
            tc.tile_pool(name="const", bufs=1) as constp,
            tc.tile_pool(name="gate", bufs=1) as gatep,
            tc.tile_pool(name="wpool", bufs=wbufs) as wp,
            tc.tile_pool(name="hpool", bufs=hbufs) as hp,
            tc.tile_pool(name="apool", bufs=abufs) as ap_,
            tc.tile_pool(name="spool", bufs=2) as sp,
            tc.tile_pool(name="psum1", bufs=2, space="PSUM") as pp,
            tc.tile_pool(name="psum2", bufs=2, space="PSUM") as pp2,
            tc.tile_pool(name="psumg", bufs=2, space="PSUM") as ppg,
            tc.tile_pool(name="dram", bufs=1, space="DRAM") as dp,
        ):
            ident = constp.tile([P, P], FP32)
            make_identity(nc, ident[:])
            if use_b2:
                ones1 = constp.tile([1, P], BF16)
                nc.vector.memset(ones1[:], 1.0)

            def body(_i=None):
                tiles_last = [t for t in range(NTT) if rsegs[t] == E - 1]
                a_dram = dp.tile([R_pad, D], BF16, tag="a_tab")

                # ---------- input loads (few, large, pre-swizzled) ----------
                # x columns of segment 0 first so fc1 can start immediately
                xsb = gatep.tile([P, DC * R], BF16, tag="xsb")
                c0 = caps[0]
                nc.sync.dma_start(
                    out=xsb[:].rearrange("p (d r) -> p d r", r=R)[:, :, 0:c0],
                    in_=xt_d[:].rearrange("p (d r) -> p d r", r=R)[:, :, 0:c0],
                )
                # segment-0 fc1 weights in quarter slices so matmuls start early
                wsb = {}
                wsb[0] = wp.tile([P, W1C + W2C], BF16, tag="w", name="wsb0")
                for q in range(4):
                    nc.sync.dma_start(
                        out=wsb[0][:, q * (W1C // 4) : (q + 1) * (W1C // 4)],
                        in_=w_d[0][:, q * (W1C // 4) : (q + 1) * (W1C // 4)],
                    )
                gfsb = gatep.tile([P, FC * NS], BF16, tag="gfsb")
                nc.sync.dma_start(out=gfsb[:], in_=gft_d[:])
                wgwn = gatep.tile([P, FC * 2 * E], BF16, tag="wgwn")
                nc.sync.dma_start(out=wgwn[:], in_=wgwn_d[:])
                # rest of x
                nc.sync.dma_start(
                    out=xsb[:].rearrange("p (d r) -> p d r", r=R)[:, :, c0:R],
                    in_=xt_d[:].rearrange("p (d r) -> p d r", r=R)[:, :, c0:R],
                )
                wsb[1] = wp.tile([P, W1C + W2C], BF16, tag="w", name="wsb1")
                nc.sync.dma_start(out=wsb[1][:, :W1C], in_=w_d[1][:, :W1C])
                nc.sync.dma_start(out=wsb[0][:, W1C:], in_=w_d[0][:, W1C:])
                nc.sync.dma_start(out=wsb[1][:, W1C:], in_=w_d[1][:, W1C:])
                nssb = gatep.tile([E, NS], FP32, tag="nssb")
                nc.sync.dma_start(out=nssb[:], in_=nst_d[:])
                j12sb = gatep.tile([P, 2 * NTT], mybir.dt.int32, tag="j12sb")
                smsb = gatep.tile([P, 8], FP32, tag="smsb")
                if timing:
                    # internal j tensor holds garbage; keep gather rows at 0
                    nc.vector.memset(j12sb[:], 0)
                    nc.vector.memset(smsb[:], 1.0)
                else:
                    nc.sync.dma_start(out=j12sb[:], in_=j12_d[:])
                    nc.sync.dma_start(out=smsb[:], in_=sm_d[:])
                if use_b1:
                    b1sb = gatep.tile([P, E * HC], FP32, tag="b1sb")
                    nc.sync.dma_start(out=b1sb[:], in_=b1_d[:])
                if use_b2:
                    b2sb = gatep.tile([E, D], BF16, tag="b2sb")
                    nc.sync.dma_start(out=b2sb[:], in_=b2_d[:])

                hsb = {}
                ssb = gatep.tile([P, NTT * D], BF16, tag="ssb")
                lg_sb = gatep.tile([E, NS], FP32, tag="lg")
                lgc_sb = gatep.tile([E, NS], FP32, tag="lgc")
                lgn_sb = gatep.tile([E, NS], FP32, tag="lgn")
                g1sb = gatep.tile([P, NTT], FP32, tag="g1")
                g2sb = gatep.tile([P, NTT], FP32, tag="g2")
                gab = gatep.tile([P, 4 * max(1, len(tiles_last))], FP32,
                                 tag="gab")
                if "gate" in skip:
                    nc.vector.memset(g1sb[:], 0.5)
                    nc.vector.memset(g2sb[:], 0.5)
                    nc.vector.memset(gab[:], 0.5)

                def emit_fc1(k):
                    cap = caps[k]
                    off = int(offs[k])
                    hsb[k] = hp.tile([P, HC * cap], BF16, tag="h", name=f"hsb{k}")
                    for hp2 in range(HC // 2 if "fc1" not in skip else 1):
                        ps = pp.tile([P, 1024], FP32, tag="fc1_ps")
                        for h2 in range(2):
                            h = hp2 * 2 + h2
                            for d in range(DC):
                                nc.tensor.matmul(
                                    ps[:, h2 * 512 : h2 * 512 + cap],
                                    lhsT=wsb[k][:, h * 512 + d * P : h * 512 + (d + 1) * P],
                                    rhs=xsb[:, d * R + off : d * R + off + cap],
                                    start=(d == 0),
                                    stop=(d == DC - 1),
                                )
                        if use_b1:
                            for h2 in range(2):
                                h = hp2 * 2 + h2
                                nc.scalar.activation(
                                    hsb[k][:, h * cap : (h + 1) * cap],
                                    ps[:, h2 * 512 : h2 * 512 + cap],
                                    AF.Gelu,
                                    bias=b1sb[:, k * HC + h : k * HC + h + 1],
                                )
                        else:
                            nc.scalar.activation(
                                hsb[k][:].rearrange("p (h n) -> p h n", n=cap)[
                                    :, hp2 * 2 : hp2 * 2 + 2, :
                                ],
                                ps[:].rearrange("p (h n) -> p h n", n=512)[
                                    :, :, 0:cap
                                ],
                                AF.Gelu,
                            )

                def emit_gating_mm():
                    # clean and noise logits into SBUF via small psum chunks
                    for t in range(2):
                        t0 = t * 512
                        for j, dst in ((0, lgc_sb), (E, lgn_sb)):
                            gps = ppg.tile([E, 512], FP32, tag="gate_ps")
                            for c in range(FC):
                                nc.tensor.matmul(
                                    gps[:],
                                    lhsT=wgwn[:, c * 2 * E + j : c * 2 * E + j + E],
                                    rhs=gfsb[:, c * NS + t0 : c * NS + t0 + 512],
                                    start=(c == 0),
                                    stop=(c == FC - 1),
                                )
                            nc.vector.tensor_copy(dst[:, t0 : t0 + 512], gps[:])

                def emit_gate_softplus():
                    # stddev = softplus(noise_logits) + 1e-2; logits = clean + noise*std
                    nc.scalar.activation(lgn_sb[:], lgn_sb[:], AF.Exp)
                    nc.vector.tensor_scalar_add(lgn_sb[:], lgn_sb[:], 1.0)
                    nc.scalar.activation(lgn_sb[:], lgn_sb[:], AF.Ln)
                    nc.vector.tensor_scalar_add(lgn_sb[:], lgn_sb[:], 1e-2)
                    nc.vector.tensor_mul(lgn_sb[:], lgn_sb[:], nssb[:])
                    nc.vector.tensor_add(lg_sb[:], lgn_sb[:], lgc_sb[:])

                def emit_gating_post():
                    # transpose to [tok, E] tiles; top-2 softmax via sigmoid
                    trp = ppg.tile([P, NTT * E], FP32, tag="gate_ps")
                    for t in range(NTT):
                        nc.tensor.transpose(
                            trp[:, t * E : (t + 1) * E],
                            lg_sb[:, t * P : (t + 1) * P],
                            ident[:E, :E],
                        )
                    lt8 = sp.tile([P, NTT * E], FP32, tag="lt8")
                    nc.vector.tensor_copy(lt8[:], trp[:])
                    mx8 = sp.tile([P, NTT * 8], FP32, tag="mx8")
                    d21 = sp.tile([P, NTT], FP32, tag="d21")
                    e21 = sp.tile([P, NTT], FP32, tag="e21")
                    t1g = sp.tile([P, NTT], FP32, tag="t1g")
                    for t in range(NTT):
                        nc.vector.max(
                            out=mx8[:, t * 8 : (t + 1) * 8],
                            in_=lt8[:, t * E : (t + 1) * E],
                        )
                        nc.vector.tensor_sub(
                            d21[:, t : t + 1],
                            mx8[:, t * 8 + 1 : t * 8 + 2],
                            mx8[:, t * 8 : t * 8 + 1],
                        )
                    # g1 = sigmoid(v1-v2) = 1/(1+e), g2 = 1-g1 = g1*e
                    nc.scalar.activation(e21[:], d21[:], AF.Exp)
                    nc.vector.tensor_scalar_add(t1g[:], e21[:], 1.0)
                    nc.vector.reciprocal(g1sb[:], t1g[:])
                    nc.vector.tensor_mul(g2sb[:], g1sb[:], e21[:])

                def emit_fc2(k):
                    cap = caps[k]
                    ntt = ntts[k]
                    aout = ap_.tile([P, ntt * D], BF16, tag="aout")
                    for tt in range(ntt):
                        m = min(P, cap - tt * P)
                        ps2 = pp2.tile([P, D], FP32, tag="fc2_ps")
                        for h in range(HC if "fc2" not in skip else 1):
                            nc.tensor.matmul(
                                ps2[:m],
                                lhsT=hsb[k][:, h * cap + tt * P : h * cap + tt * P + m],
                                rhs=wsb[k][:, W1C + h * D : W1C + (h + 1) * D],
                                start=(h == 0),
                                stop=(h == (HC if "fc2" not in skip else 1) - 1
                                      and not use_b2),
                            )
                        if use_b2:
                            nc.tensor.matmul(
                                ps2[:m],
                                lhsT=ones1[:, :m],
                                rhs=b2sb[k : k + 1, :],
                                start=False,
                                stop=True,
                            )
                        # full-tile exp (pad rows hold stale-but-finite psum)
                        nc.scalar.activation(
                            aout[:, tt * D : (tt + 1) * D], ps2[:], AF.Exp
                        )
                    del hsb[k]
                    if k == E - 1:
                        # last segment's rows are pinned per final-tile token
                        # slot; combines read them straight from SBUF
                        aout_last.append(aout)
                    elif "astore" not in skip:
                        nc.sync.dma_start(
                            out=a_dram[offp[k] : offp[k + 1]].rearrange(
                                "(t p) j -> p t j", p=P
                            ),
                            in_=aout[:].rearrange("p (t j) -> p t j", j=D),
                        )

                def emit_combine(t):
                    # gate-weighted sum parked into ssb; Ln + y output happen
                    # once at the end so ACT never blocks the gelu pipeline.
                    # both expert rows come from one paired indirect gather.
                    if rsegs[t] == E - 1:
                        tt0 = t - (NTT - ntts[E - 1])
                        pref = int(offp[E - 1]) + (tt0 + 1) * P
                    else:
                        pref = int(offp[rsegs[t] + 1])
                    b12 = sp.tile([P, 2 * D], BF16, tag="b12")
                    nc.gpsimd.indirect_dma_start(
                        out=b12[:, :D],
                        out_offset=None,
                        in_=a_dram[0:pref, :],
                        in_offset=bass.IndirectOffsetOnAxis(
                            ap=j12sb[:, 2 * t : 2 * t + 1], axis=0
                        ),
                    )
                    nc.gpsimd.indirect_dma_start(
                        out=b12[:, D:],
                        out_offset=None,
                        in_=a_dram[0:pref, :],
                        in_offset=bass.IndirectOffsetOnAxis(
                            ap=j12sb[:, 2 * t + 1 : 2 * t + 2], axis=0
                        ),
                    )
                    s2 = sp.tile([P, D], FP32, tag="s2")
                    nc.vector.tensor_scalar_mul(
                        ssb[:, t * D : (t + 1) * D], b12[:, :D], g1sb[:, t : t + 1]
                    )
                    nc.vector.tensor_scalar_mul(s2[:], b12[:, D:], g2sb[:, t : t + 1])
                    nc.vector.tensor_add(
                        ssb[:, t * D : (t + 1) * D],
                        ssb[:, t * D : (t + 1) * D],
                        s2[:],
                    )

                def emit_gab():
                    # gA = g1 + s*(g2-g1), gB = (g1+g2) - gA for the last tiles
                    for i, t in enumerate(tiles_last):
                        tt0 = t - (NTT - ntts[E - 1])
                        nc.vector.tensor_sub(gab[:, 4 * i + 2 : 4 * i + 3],
                                             g2sb[:, t : t + 1],
                                             g1sb[:, t : t + 1])
                        nc.vector.tensor_mul(gab[:, 4 * i + 2 : 4 * i + 3],
                                             gab[:, 4 * i + 2 : 4 * i + 3],
                                             smsb[:, 4 * tt0 : 4 * tt0 + 1])
                        nc.vector.tensor_add(gab[:, 4 * i : 4 * i + 1],
                                             g1sb[:, t : t + 1],
                                             gab[:, 4 * i + 2 : 4 * i + 3])
                        nc.vector.tensor_add(gab[:, 4 * i + 3 : 4 * i + 4],
                                             g1sb[:, t : t + 1],
                                             g2sb[:, t : t + 1])
                        nc.vector.tensor_sub(gab[:, 4 * i + 1 : 4 * i + 2],
                                             gab[:, 4 * i + 3 : 4 * i + 4],
                                             gab[:, 4 * i : 4 * i + 1])

                def emit_combine_last(t):
                    # side A: the non-last-segment row, gathered early (rows
                    # all sit below offp[E-1]); side B: the pinned last-segment
                    # row read directly from the aout SBUF tile, with a
                    # gathered fallback for tokens routed entirely below
                    tt0 = t - (NTT - ntts[E - 1])
                    direct = aout_last[0][:, tt0 * D : (tt0 + 1) * D]
                    pref = int(offp[E - 1])
                    bA = sp.tile([P, D], BF16, tag="bA")
                    nc.gpsimd.indirect_dma_start(
                        out=bA[:],
                        out_offset=None,
                        in_=a_dram[0:pref, :],
                        in_offset=bass.IndirectOffsetOnAxis(
                            ap=j12sb[:, 2 * t : 2 * t + 1], axis=0
                        ),
                    )
                    i = tiles_last.index(t)
                    gA = gab[:, 4 * i : 4 * i + 1]
                    gB = gab[:, 4 * i + 1 : 4 * i + 2]
                    s2 = sp.tile([P, D], FP32, tag="s2")
                    if t == tiles_last[0]:
                        # blend in the straggler fallback rows (tile 0 only)
                        bS = sp.tile([P, D], BF16, tag="bS")
                        nc.gpsimd.indirect_dma_start(
                            out=bS[:],
                            out_offset=None,
                            in_=a_dram[0:pref, :],
                            in_offset=bass.IndirectOffsetOnAxis(
                                ap=j12sb[:, 2 * t + 1 : 2 * t + 2], axis=0
                            ),
                        )
                        bB = sp.tile([P, D], FP32, tag="bB")
                        nc.vector.tensor_sub(bB[:], direct, bS[:])
                        nc.vector.tensor_scalar_mul(
                            bB[:], bB[:], smsb[:, 4 * tt0 + 1 : 4 * tt0 + 2]
                        )
                        nc.vector.tensor_add(bB[:], bB[:], bS[:])
                        nc.scalar.activation(s2[:], bB[:], AF.Copy, scale=gB)
                    else:
                        nc.scalar.activation(s2[:], direct, AF.Copy, scale=gB)
                    nc.vector.tensor_scalar_mul(
                        ssb[:, t * D : (t + 1) * D], bA[:], gA
                    )
                    nc.vector.tensor_add(
                        ssb[:, t * D : (t + 1) * D],
                        ssb[:, t * D : (t + 1) * D],
                        s2[:],
                    )

                def emit_ln_y(t0, t1):
                    # ln over parked tiles [t0, t1) and the matching y store
                    if t1 <= t0:
                        return
                    nc.scalar.activation(
                        ssb[:, t0 * D : t1 * D], ssb[:, t0 * D : t1 * D], AF.Ln
                    )
                    nc.sync.dma_start(
                        out=y_d[t0 * P : t1 * P].rearrange("(t p) j -> p t j", p=P),
                        in_=ssb[:, t0 * D : t1 * D].rearrange(
                            "p (t j) -> p t j", j=D
                        ),
                    )

                # early tiles are parked before the last group starts; their
                # Ln+y run in the last group's fc2 slack, rest at the end
                nsplit = sum(1 for t in range(NTT) if rsegs[t] <= E - 3)
                aout_last = []

                # ---------- main schedule: groups of QS segments ----------
                for g in range(E // QS):
                    k0, k1 = g * QS, g * QS + QS - 1
                    for k in range(k0, k1 + 1):
                        emit_fc1(k)
                        if k == 0 and "gate" not in skip:
                            emit_gating_mm()
                    if g == 0 and "gate" not in skip:
                        emit_gate_softplus()
                    # prefetch next group's weights as whole-segment DMAs,
                    # interleaved with the fc2s so each group's a-table store
                    # never queues behind both 4MB transfers in the DMA FIFO
                    def prefetch(kn):
                        wsb[kn] = wp.tile([P, W1C + W2C], BF16, tag="w",
                                          name=f"wsb{kn}")
                        if "wdma" in skip:
                            nc.sync.dma_start(out=wsb[kn][:, :128],
                                              in_=w_d[kn][:, :128])
                        else:
                            nc.sync.dma_start(out=wsb[kn][:], in_=w_d[kn][:])
                    kns = list(range(k1 + 1, min(k1 + 1 + QS, E)))
                    if kns:
                        prefetch(kns[0])
                    emit_fc2(k0)
                    del wsb[k0]
                    for t in range(NTT):
                        if rsegs[t] == k0 and "combine" not in skip:
                            emit_combine(t)
                    if g == E // QS - 1:
                        emit_ln_y(0, nsplit)
                    for kn in kns[1:]:
                        prefetch(kn)
                    emit_fc2(k1)
                    del wsb[k1]
                    for t in range(NTT):
                        if rsegs[t] == k1 and "combine" not in skip:
                            if k1 == E - 1:
                                emit_combine_last(t)
                            else:
                                emit_combine(t)
                    if g == 0 and "gate" not in skip:
                        emit_gating_post()
                        emit_gab()
                if "combine" not in skip:
                    nltail = min(tiles_last) if tiles_last else NTT
                    if nltail > nsplit:
                        emit_ln_y(nsplit, nltail)
                    emit_ln_y(nltail, NTT)

            if reps > 1:
                with tc.For_i(0, reps, 1):
                    body()
            else:
                body()
            if timing:
                nc.sync.dma_start(out=yo_d[:], in_=ident[:1, :4])

    nc.compile()
    return nc


def _route(gate_feat, noise, w_gate, w_noise):
    """Host-side routing structure (fp32 numpy, matches jax top-k selection)."""
    clean = gate_feat @ w_gate
    stddev = np.logaddexp(gate_feat @ w_noise, 0.0) + np.float32(1e-2)
    logits = clean.astype(np.float32) + noise * stddev.astype(np.float32)
    top2 = np.argsort(-logits, axis=1, kind="stable")[:, :TOPK].astype(np.int32)
    return top2


def _prepare(x, gate_feat, noise, w_gate, w_noise, fc1_w, fc1_b, fc2_w, fc2_b):
    x = np.asarray(x, dtype=np.float32)
    gate_feat = np.asarray(gate_feat, dtype=np.float32)
    noise = np.asarray(noise, dtype=np.float32)
    bf = ml_dtypes.bfloat16

    top2 = _route(gate_feat, noise, w_gate, w_noise)

    # deal tokens to cores round-robin within each expert-pair class so all
    # cores see a near-identical per-expert load
    pairs = np.sort(top2, axis=1)
    key = pairs[:, 0] * E + pairs[:, 1]
    sidx = np.argsort(key, kind="stable")
    toks = [sidx[c::NC] for c in range(NC)]

    cnt_global = np.bincount(top2.ravel(), minlength=E)
    order = np.argsort(-cnt_global, kind="stable").astype(np.int64)
    seg_of_expert = np.empty(E, dtype=np.int64)
    seg_of_expert[order] = np.arange(E)

    core_meta = []
    caps = np.zeros(E, dtype=np.int64)
    for c in range(NC):
        t2 = top2[toks[c]]
        pair_seg = seg_of_expert[t2.ravel()]
        caps = np.maximum(caps, np.bincount(pair_seg, minlength=E))
        core_meta.append(pair_seg)

    # pin the last (smallest) segment to exactly two fixed tiles: its rows sit
    # at slot (perm position - (NS - 2P)) so the final token tiles can read
    # them straight from the producing SBUF tile (no store+gather round trip)
    last_pin = bool(caps[E - 1] <= 2 * P) and all(
        np.sum(m == E - 1) >= P for m in core_meta
    )
    if last_pin:
        caps[E - 1] = 2 * P
    ntts = (caps + P - 1) // P
    offs = np.concatenate([[0], np.cumsum(caps)]).astype(np.int64)
    offp = np.concatenate([[0], np.cumsum(ntts * P)]).astype(np.int64)
    R = int(offs[-1])

    # ---- pass 1: per-core routing rows / readiness ----
    percore = []
    rsegs_cores = []
    for c in range(NC):
        pair_seg = core_meta[c]
        sort_idx = np.argsort(pair_seg, kind="stable")
        seg_counts = np.bincount(pair_seg, minlength=E)
        seg_starts = np.concatenate([[0], np.cumsum(seg_counts)])
        pos_in_seg = np.arange(2 * NS) - seg_starts[pair_seg[sort_idx]]
        rows_sorted = offp[pair_seg[sort_idx]] + pos_in_seg
        rows_of_pair = np.empty(2 * NS, dtype=np.int64)
        rows_of_pair[sort_idx] = rows_sorted

        # readiness: last segment a token's pair rows land in; sort tokens so
        # early-ready tokens combine while later segments still compute
        ready = np.maximum(pair_seg[0::2], pair_seg[1::2])
        perm = np.argsort(ready, kind="stable")
        rsegs_cores.append(ready[perm].reshape(NTT, P).max(axis=1))

        if last_pin:
            tokpos = np.empty(NS, dtype=np.int64)
            tokpos[perm] = np.arange(NS)
            lastmask = pair_seg == E - 1
            tp = tokpos[np.arange(2 * NS) // 2][lastmask]
            assert np.all(tp >= NS - 2 * P), "last-seg rows must sit in final tiles"
            rows_of_pair[lastmask] = offp[E - 1] + (tp - (NS - 2 * P))
        percore.append((pair_seg, sort_idx, seg_counts, seg_starts,
                        rows_of_pair, ready, perm))

    rsegs = tuple(int(v) for v in np.max(np.stack(rsegs_cores), axis=0))
    tiles_last = [t for t in range(NTT) if rsegs[t] == E - 1] if last_pin else []

    # ---- shared weight packing (global segment order) ----
    w1p = np.transpose(
        np.asarray(fc1_w).reshape(E, HC, P, DC, P), (0, 4, 1, 3, 2)
    ).reshape(E, P, W1C)
    w2p = np.transpose(
        np.asarray(fc2_w).reshape(E, D, HC, P), (0, 3, 2, 1)
    ).reshape(E, P, W2C)
    w_all = np.ascontiguousarray(
        np.concatenate([w1p, w2p], axis=2)[order]
    ).astype(bf)

    wg_r = np.asarray(w_gate, np.float32).reshape(FC, P, E)
    wn_r = np.asarray(w_noise, np.float32).reshape(FC, P, E)
    wgwn = np.ascontiguousarray(
        np.transpose(np.concatenate([wg_r, wn_r], axis=2), (1, 0, 2)).reshape(
            P, FC * 2 * E
        )
    ).astype(bf)

    use_b1 = bool(np.any(np.asarray(fc1_b)))
    use_b2 = bool(np.any(np.asarray(fc2_b)))
    b1p = np.ascontiguousarray(
        np.transpose(np.asarray(fc1_b, np.float32)[order].reshape(E, HC, P),
                     (2, 0, 1)).reshape(P, E * HC)
    )
    b2p = np.ascontiguousarray(np.asarray(fc2_b)[order]).astype(bf)

    # ---- pass 2: per-core input packing ----
    in_maps = []
    gidxs = []
    for c in range(NC):
        (pair_seg, sort_idx, seg_counts, seg_starts,
         rows_of_pair, ready, perm) = percore[c]
        j1 = rows_of_pair[0::2].astype(np.int32)
        j2 = rows_of_pair[1::2].astype(np.int32)

        # x columns in segment order, padded per segment (unpadded offsets)
        tok_sorted = sort_idx // 2
        cols = np.zeros(R, dtype=np.int64)
        for k in range(E):
            s0, n = int(seg_starts[k]), int(seg_counts[k])
            cols[offs[k] : offs[k] + n] = tok_sorted[s0 : s0 + n]
        if last_pin:
            cols[offs[E - 1] : offs[E - 1] + 2 * P] = perm[NS - 2 * P :]
        x_loc = x[toks[c]]
        xt = np.ascontiguousarray(
            np.transpose(x_loc[cols].reshape(R, DC, P), (2, 1, 0)).reshape(
                P, DC * R
            )
        ).astype(bf)

        gf = gate_feat[toks[c]][perm]
        gft = np.ascontiguousarray(
            np.transpose(gf.reshape(NS, FC, P), (2, 1, 0)).reshape(P, FC * NS)
        ).astype(bf)
        nst = np.ascontiguousarray(noise[toks[c]][perm].T.astype(np.float32))

        # j12 pairs adjacent: col 2t = top-1 row, 2t+1 = top-2 row of tile t.
        # tiles handled by combine_last get (jA = non-last-seg row, jB2 =
        # straggler fallback row) plus the s/m selectors in sm
        j1t = j1[perm].reshape(NTT, P)
        j2t = j2[perm].reshape(NTT, P)
        s71 = (pair_seg[0::2] == E - 1)[perm].reshape(NTT, P)
        s72 = (pair_seg[1::2] == E - 1)[perm].reshape(NTT, P)
        sm = np.zeros((P, 8), dtype=np.float32)
        for i, t in enumerate(tiles_last):
            has7 = s71[t] | s72[t]
            if t != tiles_last[0]:
                assert has7.all(), "stragglers must sit in the first last-tile"
            ja = np.where(s71[t], j2t[t], j1t[t])
            jb2 = np.where(has7, 0, j2t[t])
            j1t[t], j2t[t] = ja, jb2
            tt0 = t - (NTT - int(ntts[E - 1]))
            sm[:, 4 * tt0 + 0] = s71[t].astype(np.float32)
            sm[:, 4 * tt0 + 1] = has7.astype(np.float32)
        j12 = np.ascontiguousarray(
            np.stack([j1t.T, j2t.T], axis=2).reshape(P, 2 * NTT)
        )
        im = {
            "xt": xt,
            "gft": gft,
            "nst": nst,
            "wgwn": wgwn,
            "w": w_all,
            "j12": j12,
            "sm": sm,
        }
        if use_b1:
            im["b1"] = b1p
        if use_b2:
            im["b2"] = b2p
        in_maps.append(im)
        gidxs.append(toks[c][perm])

    return tuple(int(v) for v in caps), rsegs, gidxs, in_maps


def kernel(x, gate_feat, noise, w_gate, w_noise, fc1_w, fc1_b, fc2_w, fc2_b,
           _reps=1):
    caps, rsegs, gidxs, in_maps = _prepare(
        x, gate_feat, noise, w_gate, w_noise, fc1_w, fc1_b, fc2_w, fc2_b
    )
    use_b1 = "b1" in in_maps[0]
    use_b2 = "b2" in in_maps[0]
    key = (caps, rsegs, int(_reps), use_b1, use_b2)
    if key not in _nc_cache:
        _nc_cache[key] = _build_nc(caps, rsegs, reps=_reps, use_b1=use_b1,
                                   use_b2=use_b2)
    nc = _nc_cache[key]
    try:
        res = run_bass_kernel_spmd(nc, in_maps, core_ids=list(range(NC)))
    except Exception:
        # transient device wedge (seen once as NRT_EXEC_UNIT_UNRECOVERABLE on a
        # cold device); one retry after the runtime recovers
        res = run_bass_kernel_spmd(nc, in_maps, core_ids=list(range(NC)))
    y = np.empty((N, D), np.float32)
    for c in range(NC):
        y[gidxs[c]] = res.results[c]["y"]
    return y


# revision 47
# speedup vs baseline: 1.1369x; 1.1151x over previous
"""MoE (noisy top-2-of-8 gating) Trainium2 kernel.

Strategy: data-parallel over tokens (1024/core on 8 cores). The host computes
routing structure only (which expert each token goes to — the sharding
metadata); all FLOPs (gating values, expert MLPs, combine) run on device.

Tokens are dealt to cores round-robin within each (expert,expert) routing
class so every core sees a near-identical per-expert distribution — the SPMD
per-segment capacity (max over cores) then sits close to the global mean.
Per core the tokens are permuted into 8 expert segments (globally ordered by
descending count). All inputs are host-packed into exact SBUF layout so every
load is one large contiguous DMA (per-DMA fixed costs dominate small
transfers). Expert segments are processed in groups of two: fc1 (+exact-erf
GELU, batched two hidden-chunks per activation op) into hidden-major h tiles,
then fc2 with h-stationary matmuls, exp straight out of PSUM, one store per
segment into a padded-row DRAM exp-table, and the top-2 combine via
indirect-DMA row gathers + per-partition gate scaling + Ln.
"""

import numpy as np
import ml_dtypes

import concourse.bacc as bacc
import concourse.bass as bass
import concourse.mybir as mybir
import concourse.tile as tile
from concourse.bass_utils import run_bass_kernel_spmd
from concourse.masks import make_identity

BF16 = mybir.dt.bfloat16
FP32 = mybir.dt.float32
AF = mybir.ActivationFunctionType

N, D, H, E, TOPK = 8192, 512, 2048, 8, 2
NC = 8
NS = N // NC          # tokens per core
P = 128
NTT = NS // P         # token tiles per core (8)
DC = D // P           # d chunks (4)
HC = H // P           # hidden chunks (16)
FC = (2 * D) // P     # gate feature chunks (8)
W1C = DC * H          # w1 SBUF cols (8192)
W2C = HC * D          # w2 SBUF cols (8192)

_nc_cache: dict = {}


def _build_nc(caps, rsegs=(7,) * 8, reps=1, timing=False, use_b1=False,
              use_b2=False, wbufs=3, hbufs=3, abufs=3, QS=2, skip=(),
              pfint=True):
    """Build the SPMD Bass program for per-segment capacities `caps`.

    timing=True makes all data tensors internal DRAM (no host transfer) and
    the output a dummy, so repeated-execution wall-clock isolates device time.
    """
    caps = tuple(int(c) for c in caps)
    R = sum(caps)
    offs = np.concatenate([[0], np.cumsum(caps)]).astype(int)
    ntts = [(c + P - 1) // P for c in caps]
    offp = np.concatenate([[0], np.cumsum([n * P for n in ntts])]).astype(int)
    R_pad = int(offp[-1])

    nc = bacc.Bacc("TRN2", target_bir_lowering=False, debug=False)

    if timing:
        def param(name, shape, dtype):
            return nc.dram_tensor(name, shape, dtype)
        nc.declare_dram_parameter("tdin", [1, 4], FP32, isOutput=False)
        y_d = nc.dram_tensor("y", [NS, D], BF16)
        yo_d = nc.declare_dram_parameter("yo", [1, 4], FP32, isOutput=True)
    else:
        def param(name, shape, dtype):
            return nc.declare_dram_parameter(name, shape, dtype, isOutput=False)
        y_d = nc.declare_dram_parameter("y", [NS, D], BF16, isOutput=True)

    xt_d = param("xt", [P, DC * R], BF16)
    gft_d = param("gft", [P, FC * NS], BF16)
    nst_d = param("nst", [E, NS], FP32)
    wgwn_d = param("wgwn", [P, FC * 2 * E], BF16)
    w_d = param("w", [E, P, W1C + W2C], BF16)
    j12_d = param("j12", [P, 2 * NTT], mybir.dt.int32)
    sm_d = param("sm", [P, 8], FP32)
    if use_b1:
        b1_d = param("b1", [P, E * HC], FP32)
    if use_b2:
        b2_d = param("b2", [E, D], BF16)

    with tile.TileContext(nc) as tc:
        with (
            tc.tile_pool(name="const", bufs=1) as constp,
            tc.tile_pool(name="gate", bufs=1) as gatep,
            tc.tile_pool(name="wpool", bufs=wbufs) as wp,
            tc.tile_pool(name="hpool", bufs=hbufs) as hp,
            tc.tile_pool(name="apool", bufs=abufs) as ap_,
            tc.tile_pool(name="spool", bufs=2) as sp,
            tc.tile_pool(name="psum1", bufs=2, space="PSUM") as pp,
            tc.tile_pool(name="psum2", bufs=2, space="PSUM") as pp2,
            tc.tile_pool(name="psumg", bufs=2, space="PSUM") as ppg,
            tc.tile_pool(name="dram", bufs=1, space="DRAM") as dp,
        ):
            ident = constp.tile([P, P], FP32)
            make_identity(nc, ident[:])
            if use_b2:
                ones1 = constp.tile([1, P], BF16)
                nc.vector.memset(ones1[:], 1.0)

            def body(_i=None):
                tiles_last = [t for t in range(NTT) if rsegs[t] == E - 1]
                a_dram = dp.tile([R_pad, D], BF16, tag="a_tab")

                # ---------- input loads (few, large, pre-swizzled) ----------
                # x columns of segment 0 first so fc1 can start immediately
                xsb = gatep.tile([P, DC * R], BF16, tag="xsb")
                c0 = caps[0]
                nc.sync.dma_start(
                    out=xsb[:].rearrange("p (d r) -> p d r", r=R)[:, :, 0:c0],
                    in_=xt_d[:].rearrange("p (d r) -> p d r", r=R)[:, :, 0:c0],
                )
                # segment-0 fc1 weights in quarter slices so matmuls start early
                wsb = {}
                wsb[0] = wp.tile([P, W1C + W2C], BF16, tag="w", name="wsb0")
                for q in range(4):
                    nc.sync.dma_start(
                        out=wsb[0][:, q * (W1C // 4) : (q + 1) * (W1C // 4)],
                        in_=w_d[0][:, q * (W1C // 4) : (q + 1) * (W1C // 4)],
                    )
                gfsb = gatep.tile([P, FC * NS], BF16, tag="gfsb")
                nc.sync.dma_start(out=gfsb[:], in_=gft_d[:])
                wgwn = gatep.tile([P, FC * 2 * E], BF16, tag="wgwn")
                nc.sync.dma_start(out=wgwn[:], in_=wgwn_d[:])
                # rest of x
                nc.sync.dma_start(
                    out=xsb[:].rearrange("p (d r) -> p d r", r=R)[:, :, c0:R],
                    in_=xt_d[:].rearrange("p (d r) -> p d r", r=R)[:, :, c0:R],
                )
                wsb[1] = wp.tile([P, W1C + W2C], BF16, tag="w", name="wsb1")
                nc.sync.dma_start(out=wsb[1][:, :W1C], in_=w_d[1][:, :W1C])
                nc.sync.dma_start(out=wsb[0][:, W1C:], in_=w_d[0][:, W1C:])
                nc.sync.dma_start(out=wsb[1][:, W1C:], in_=w_d[1][:, W1C:])
                nssb = gatep.tile([E, NS], FP32, tag="nssb")
                nc.sync.dma_start(out=nssb[:], in_=nst_d[:])
                j12sb = gatep.tile([P, 2 * NTT], mybir.dt.int32, tag="j12sb")
                smsb = gatep.tile([P, 8], FP32, tag="smsb")
                if timing:
                    # internal j tensor holds garbage; keep gather rows at 0
                    nc.vector.memset(j12sb[:], 0)
                    nc.vector.memset(smsb[:], 1.0)
                else:
                    nc.sync.dma_start(out=j12sb[:], in_=j12_d[:])
                    nc.sync.dma_start(out=smsb[:], in_=sm_d[:])
                if use_b1:
                    b1sb = gatep.tile([P, E * HC], FP32, tag="b1sb")
                    nc.sync.dma_start(out=b1sb[:], in_=b1_d[:])
                if use_b2:
                    b2sb = gatep.tile([E, D], BF16, tag="b2sb")
                    nc.sync.dma_start(out=b2sb[:], in_=b2_d[:])

                hsb = {}
                ssb = gatep.tile([P, NTT * D], BF16, tag="ssb")
                lg_sb = gatep.tile([E, NS], FP32, tag="lg")
                lgc_sb = gatep.tile([E, NS], FP32, tag="lgc")
                lgn_sb = gatep.tile([E, NS], FP32, tag="lgn")
                g1sb = gatep.tile([P, NTT], FP32, tag="g1")
                g2sb = gatep.tile([P, NTT], FP32, tag="g2")
                gab = gatep.tile([P, 4 * max(1, len(tiles_last))], FP32,
                                 tag="gab")
                if "gate" in skip:
                    nc.vector.memset(g1sb[:], 0.5)
                    nc.vector.memset(g2sb[:], 0.5)
                    nc.vector.memset(gab[:], 0.5)

                def emit_fc1(k):
                    cap = caps[k]
                    off = int(offs[k])
                    hsb[k] = hp.tile([P, HC * cap], BF16, tag="h", name=f"hsb{k}")
                    for hp2 in range(HC // 2 if "fc1" not in skip else 1):
                        ps = pp.tile([P, 1024], FP32, tag="fc1_ps")
                        for h2 in range(2):
                            h = hp2 * 2 + h2
                            for d in range(DC):
                                nc.tensor.matmul(
                                    ps[:, h2 * 512 : h2 * 512 + cap],
                                    lhsT=wsb[k][:, h * 512 + d * P : h * 512 + (d + 1) * P],
                                    rhs=xsb[:, d * R + off : d * R + off + cap],
                                    start=(d == 0),
                                    stop=(d == DC - 1),
                                )
                        if use_b1:
                            for h2 in range(2):
                                h = hp2 * 2 + h2
                                nc.scalar.activation(
                                    hsb[k][:, h * cap : (h + 1) * cap],
                                    ps[:, h2 * 512 : h2 * 512 + cap],
                                    AF.Gelu,
                                    bias=b1sb[:, k * HC + h : k * HC + h + 1],
                                )
                        else:
                            nc.scalar.activation(
                                hsb[k][:].rearrange("p (h n) -> p h n", n=cap)[
                                    :, hp2 * 2 : hp2 * 2 + 2, :
                                ],
                                ps[:].rearrange("p (h n) -> p h n", n=512)[
                                    :, :, 0:cap
                                ],
                                AF.Gelu,
                            )

                def emit_gating_mm():
                    # clean and noise logits into SBUF via small psum chunks
                    for t in range(2):
                        t0 = t * 512
                        for j, dst in ((0, lgc_sb), (E, lgn_sb)):
                            gps = ppg.tile([E, 512], FP32, tag="gate_ps")
                            for c in range(FC):
                                nc.tensor.matmul(
                                    gps[:],
                                    lhsT=wgwn[:, c * 2 * E + j : c * 2 * E + j + E],
                                    rhs=gfsb[:, c * NS + t0 : c * NS + t0 + 512],
                                    start=(c == 0),
                                    stop=(c == FC - 1),
                                )
                            nc.vector.tensor_copy(dst[:, t0 : t0 + 512], gps[:])

                def emit_gate_softplus():
                    # stddev = softplus(noise_logits) + 1e-2; logits = clean + noise*std
                    nc.scalar.activation(lgn_sb[:], lgn_sb[:], AF.Exp)
                    nc.vector.tensor_scalar_add(lgn_sb[:], lgn_sb[:], 1.0)
                    nc.scalar.activation(lgn_sb[:], lgn_sb[:], AF.Ln)
                    nc.vector.tensor_scalar_add(lgn_sb[:], lgn_sb[:], 1e-2)
                    nc.vector.tensor_mul(lgn_sb[:], lgn_sb[:], nssb[:])
                    nc.vector.tensor_add(lg_sb[:], lgn_sb[:], lgc_sb[:])

                def emit_gating_post():
                    # transpose to [tok, E] tiles; top-2 softmax via sigmoid
                    trp = ppg.tile([P, NTT * E], FP32, tag="gate_ps")
                    for t in range(NTT):
                        nc.tensor.transpose(
                            trp[:, t * E : (t + 1) * E],
                            lg_sb[:, t * P : (t + 1) * P],
                            ident[:E, :E],
                        )
                    lt8 = sp.tile([P, NTT * E], FP32, tag="lt8")
                    nc.vector.tensor_copy(lt8[:], trp[:])
                    mx8 = sp.tile([P, NTT * 8], FP32, tag="mx8")
                    d21 = sp.tile([P, NTT], FP32, tag="d21")
                    e21 = sp.tile([P, NTT], FP32, tag="e21")
                    t1g = sp.tile([P, NTT], FP32, tag="t1g")
                    for t in range(NTT):
                        nc.vector.max(
                            out=mx8[:, t * 8 : (t + 1) * 8],
                            in_=lt8[:, t * E : (t + 1) * E],
                        )
                        nc.vector.tensor_sub(
                            d21[:, t : t + 1],
                            mx8[:, t * 8 + 1 : t * 8 + 2],
                            mx8[:, t * 8 : t * 8 + 1],
                        )
                    # g1 = sigmoid(v1-v2) = 1/(1+e), g2 = 1-g1 = g1*e
                    nc.scalar.activation(e21[:], d21[:], AF.Exp)
                    nc.vector.tensor_scalar_add(t1g[:], e21[:], 1.0)
                    nc.vector.reciprocal(g1sb[:], t1g[:])
                    nc.vector.tensor_mul(g2sb[:], g1sb[:], e21[:])

                def emit_fc2(k):
                    cap = caps[k]
                    ntt = ntts[k]
                    aout = ap_.tile([P, ntt * D], BF16, tag="aout")
                    for tt in range(ntt):
                        m = min(P, cap - tt * P)
                        ps2 = pp2.tile([P, D], FP32, tag="fc2_ps")
                        for h in range(HC if "fc2" not in skip else 1):
                            nc.tensor.matmul(
                                ps2[:m],
                                lhsT=hsb[k][:, h * cap + tt * P : h * cap + tt * P + m],
                                rhs=wsb[k][:, W1C + h * D : W1C + (h + 1) * D],
                                start=(h == 0),
                                stop=(h == (HC if "fc2" not in skip else 1) - 1
                                      and not use_b2),
                            )
                        if use_b2:
                            nc.tensor.matmul(
                                ps2[:m],
                                lhsT=ones1[:, :m],
                                rhs=b2sb[k : k + 1, :],
                                start=False,
                                stop=True,
                            )
                        # full-tile exp (pad rows hold stale-but-finite psum)
                        nc.scalar.activation(
                            aout[:, tt * D : (tt + 1) * D], ps2[:], AF.Exp
                        )
                    del hsb[k]
                    if k == E - 1:
                        # last segment's rows are pinned per final-tile token
                        # slot; combines read them straight from SBUF
                        aout_last.append(aout)
                    elif "astore" not in skip:
                        nc.sync.dma_start(
                            out=a_dram[offp[k] : offp[k + 1]].rearrange(
                                "(t p) j -> p t j", p=P
                            ),
                            in_=aout[:].rearrange("p (t j) -> p t j", j=D),
                        )

                def emit_combine(t):
                    # gate-weighted sum parked into ssb; Ln + y output happen
                    # once at the end so ACT never blocks the gelu pipeline.
                    # both expert rows come from one paired indirect gather.
                    if rsegs[t] == E - 1:
                        tt0 = t - (NTT - ntts[E - 1])
                        pref = int(offp[E - 1]) + (tt0 + 1) * P
                    else:
                        pref = int(offp[rsegs[t] + 1])
                    b12 = sp.tile([P, 2 * D], BF16, tag="b12")
                    nc.gpsimd.indirect_dma_start(
                        out=b12[:, :D],
                        out_offset=None,
                        in_=a_dram[0:pref, :],
                        in_offset=bass.IndirectOffsetOnAxis(
                            ap=j12sb[:, 2 * t : 2 * t + 1], axis=0
                        ),
                    )
                    nc.gpsimd.indirect_dma_start(
                        out=b12[:, D:],
                        out_offset=None,
                        in_=a_dram[0:pref, :],
                        in_offset=bass.IndirectOffsetOnAxis(
                            ap=j12sb[:, 2 * t + 1 : 2 * t + 2], axis=0
                        ),
                    )
                    s2 = sp.tile([P, D], FP32, tag="s2")
                    nc.vector.tensor_scalar_mul(
                        ssb[:, t * D : (t + 1) * D], b12[:, :D], g1sb[:, t : t + 1]
                    )
                    nc.vector.tensor_scalar_mul(s2[:], b12[:, D:], g2sb[:, t : t + 1])
                    nc.vector.tensor_add(
                        ssb[:, t * D : (t + 1) * D],
                        ssb[:, t * D : (t + 1) * D],
                        s2[:],
                    )

                def emit_gab():
                    # gA = g1 + s*(g2-g1), gB = (g1+g2) - gA for the last tiles
                    for i, t in enumerate(tiles_last):
                        tt0 = t - (NTT - ntts[E - 1])
                        nc.vector.tensor_sub(gab[:, 4 * i + 2 : 4 * i + 3],
                                             g2sb[:, t : t + 1],
                                             g1sb[:, t : t + 1])
                        nc.vector.tensor_mul(gab[:, 4 * i + 2 : 4 * i + 3],
                                             gab[:, 4 * i + 2 : 4 * i + 3],
                                             smsb[:, 4 * tt0 : 4 * tt0 + 1])
                        nc.vector.tensor_add(gab[:, 4 * i : 4 * i + 1],
                                             g1sb[:, t : t + 1],
                                             gab[:, 4 * i + 2 : 4 * i + 3])
                        nc.vector.tensor_add(gab[:, 4 * i + 3 : 4 * i + 4],
                                             g1sb[:, t : t + 1],
                                             g2sb[:, t : t + 1])
                        nc.vector.tensor_sub(gab[:, 4 * i + 1 : 4 * i + 2],
                                             gab[:, 4 * i + 3 : 4 * i + 4],
                                             gab[:, 4 * i : 4 * i + 1])

                def emit_combine_last(t):
                    # side A: the non-last-segment row, gathered early (rows
                    # all sit below offp[E-1]); side B: the pinned last-segment
                    # row read directly from the aout SBUF tile, with a
                    # gathered fallback for tokens routed entirely below
                    tt0 = t - (NTT - ntts[E - 1])
                    direct = aout_last[0][:, tt0 * D : (tt0 + 1) * D]
                    pref = int(offp[E - 1])
                    bA = sp.tile([P, D], BF16, tag="bA")
                    nc.gpsimd.indirect_dma_start(
                        out=bA[:],
                        out_offset=None,
                        in_=a_dram[0:pref, :],
                        in_offset=bass.IndirectOffsetOnAxis(
                            ap=j12sb[:, 2 * t : 2 * t + 1], axis=0
                        ),
                    )
                    i = tiles_last.index(t)
                    gA = gab[:, 4 * i : 4 * i + 1]
                    gB = gab[:, 4 * i + 1 : 4 * i + 2]
                    s2 = sp.tile([P, D], FP32, tag="s2")
                    if t == tiles_last[0]:
                        # blend in the straggler fallback rows (tile 0 only)
                        bS = sp.tile([P, D], BF16, tag="bS")
                        nc.gpsimd.indirect_dma_start(
                            out=bS[:],
                            out_offset=None,
                            in_=a_dram[0:pref, :],
                            in_offset=bass.IndirectOffsetOnAxis(
                                ap=j12sb[:, 2 * t + 1 : 2 * t + 2], axis=0
                            ),
                        )
                        bB = sp.tile([P, D], FP32, tag="bB")
                        nc.vector.tensor_sub(bB[:], direct, bS[:])
                        nc.vector.tensor_scalar_mul(
                            bB[:], bB[:], smsb[:, 4 * tt0 + 1 : 4 * tt0 + 2]
                        )
                        nc.vector.tensor_add(bB[:], bB[:], bS[:])
                        nc.scalar.activation(s2[:], bB[:], AF.Copy, scale=gB)
                    else:
                        nc.scalar.activation(s2[:], direct, AF.Copy, scale=gB)
                    nc.vector.tensor_scalar_mul(
                        ssb[:, t * D : (t + 1) * D], bA[:], gA
                    )
                    nc.vector.tensor_add(
                        ssb[:, t * D : (t + 1) * D],
                        ssb[:, t * D : (t + 1) * D],
                        s2[:],
                    )

                def emit_ln_y(t0, t1):
                    # ln over parked tiles [t0, t1) and the matching y store
                    if t1 <= t0:
                        return
                    nc.scalar.activation(
                        ssb[:, t0 * D : t1 * D], ssb[:, t0 * D : t1 * D], AF.Ln
                    )
                    nc.sync.dma_start(
                        out=y_d[t0 * P : t1 * P].rearrange("(t p) j -> p t j", p=P),
                        in_=ssb[:, t0 * D : t1 * D].rearrange(
                            "p (t j) -> p t j", j=D
                        ),
                    )

                # early tiles are parked before the last group starts; their
                # Ln+y run in the last group's fc2 slack, rest at the end
                nsplit = sum(1 for t in range(NTT) if rsegs[t] <= E - 3)
                aout_last = []

                # ---------- main schedule: groups of QS segments ----------
                for g in range(E // QS):
                    k0, k1 = g * QS, g * QS + QS - 1
                    for k in range(k0, k1 + 1):
                        emit_fc1(k)
                        if k == 0 and "gate" not in skip:
                            emit_gating_mm()
                    if g == 0 and "gate" not in skip:
                        emit_gate_softplus()
                    # prefetch next group's weights: one whole-segment DMA
                    # each (prefetch slack far exceeds the transfer time);
                    # pfint splits them around fc2(k0) so the a-table store
                    # doesn't queue behind both transfers
                    def prefetch(kn):
                        wsb[kn] = wp.tile([P, W1C + W2C], BF16, tag="w",
                                          name=f"wsb{kn}")
                        if "wdma" in skip:
                            nc.sync.dma_start(out=wsb[kn][:, :128],
                                              in_=w_d[kn][:, :128])
                        else:
                            nc.sync.dma_start(out=wsb[kn][:], in_=w_d[kn][:])
                    kns = list(range(k1 + 1, min(k1 + 1 + QS, E)))
                    for kn in (kns[:1] if pfint else kns):
                        prefetch(kn)
                    emit_fc2(k0)
                    del wsb[k0]
                    if pfint:
                        for kn in kns[1:]:
                            prefetch(kn)
                    for t in range(NTT):
                        if rsegs[t] == k0 and "combine" not in skip:
                            emit_combine(t)
                    if g == E // QS - 1:
                        emit_ln_y(0, nsplit)
                    emit_fc2(k1)
                    del wsb[k1]
                    for t in range(NTT):
                        if rsegs[t] == k1 and "combine" not in skip:
                            if k1 == E - 1:
                                emit_combine_last(t)
                            else:
                                emit_combine(t)
                    if g == 0 and "gate" not in skip:
                        emit_gating_post()
                        emit_gab()
                if "combine" not in skip:
                    nltail = min(tiles_last) if tiles_last else NTT
                    if nltail > nsplit:
                        emit_ln_y(nsplit, nltail)
                    emit_ln_y(nltail, NTT)

            if reps > 1:
                with tc.For_i(0, reps, 1):
                    body()
            else:
                body()
            if timing:
                nc.sync.dma_start(out=yo_d[:], in_=ident[:1, :4])

    nc.compile()
    return nc


def _route(gate_feat, noise, w_gate, w_noise):
    """Host-side routing structure (fp32 numpy, matches jax top-k selection)."""
    clean = gate_feat @ w_gate
    stddev = np.logaddexp(gate_feat @ w_noise, 0.0) + np.float32(1e-2)
    logits = clean.astype(np.float32) + noise * stddev.astype(np.float32)
    top2 = np.argsort(-logits, axis=1, kind="stable")[:, :TOPK].astype(np.int32)
    return top2


def _prepare(x, gate_feat, noise, w_gate, w_noise, fc1_w, fc1_b, fc2_w, fc2_b):
    x = np.asarray(x, dtype=np.float32)
    gate_feat = np.asarray(gate_feat, dtype=np.float32)
    noise = np.asarray(noise, dtype=np.float32)
    bf = ml_dtypes.bfloat16

    top2 = _route(gate_feat, noise, w_gate, w_noise)

    # deal tokens to cores round-robin within each expert-pair class so all
    # cores see a near-identical per-expert load
    pairs = np.sort(top2, axis=1)
    key = pairs[:, 0] * E + pairs[:, 1]
    sidx = np.argsort(key, kind="stable")
    toks = [sidx[c::NC] for c in range(NC)]

    cnt_global = np.bincount(top2.ravel(), minlength=E)
    order = np.argsort(-cnt_global, kind="stable").astype(np.int64)
    seg_of_expert = np.empty(E, dtype=np.int64)
    seg_of_expert[order] = np.arange(E)

    core_meta = []
    caps = np.zeros(E, dtype=np.int64)
    for c in range(NC):
        t2 = top2[toks[c]]
        pair_seg = seg_of_expert[t2.ravel()]
        caps = np.maximum(caps, np.bincount(pair_seg, minlength=E))
        core_meta.append(pair_seg)

    # pin the last (smallest) segment to exactly two fixed tiles: its rows sit
    # at slot (perm position - (NS - 2P)) so the final token tiles can read
    # them straight from the producing SBUF tile (no store+gather round trip)
    last_pin = bool(caps[E - 1] <= 2 * P) and all(
        np.sum(m == E - 1) >= P for m in core_meta
    )
    if last_pin:
        caps[E - 1] = 2 * P
    ntts = (caps + P - 1) // P
    offs = np.concatenate([[0], np.cumsum(caps)]).astype(np.int64)
    offp = np.concatenate([[0], np.cumsum(ntts * P)]).astype(np.int64)
    R = int(offs[-1])

    # ---- pass 1: per-core routing rows / readiness ----
    percore = []
    rsegs_cores = []
    for c in range(NC):
        pair_seg = core_meta[c]
        sort_idx = np.argsort(pair_seg, kind="stable")
        seg_counts = np.bincount(pair_seg, minlength=E)
        seg_starts = np.concatenate([[0], np.cumsum(seg_counts)])
        pos_in_seg = np.arange(2 * NS) - seg_starts[pair_seg[sort_idx]]
        rows_sorted = offp[pair_seg[sort_idx]] + pos_in_seg
        rows_of_pair = np.empty(2 * NS, dtype=np.int64)
        rows_of_pair[sort_idx] = rows_sorted

        # readiness: last segment a token's pair rows land in; sort tokens so
        # early-ready tokens combine while later segments still compute
        ready = np.maximum(pair_seg[0::2], pair_seg[1::2])
        perm = np.argsort(ready, kind="stable")
        rsegs_cores.append(ready[perm].reshape(NTT, P).max(axis=1))

        if last_pin:
            tokpos = np.empty(NS, dtype=np.int64)
            tokpos[perm] = np.arange(NS)
            lastmask = pair_seg == E - 1
            tp = tokpos[np.arange(2 * NS) // 2][lastmask]
            assert np.all(tp >= NS - 2 * P), "last-seg rows must sit in final tiles"
            rows_of_pair[lastmask] = offp[E - 1] + (tp - (NS - 2 * P))
        percore.append((pair_seg, sort_idx, seg_counts, seg_starts,
                        rows_of_pair, ready, perm))

    rsegs = tuple(int(v) for v in np.max(np.stack(rsegs_cores), axis=0))
    tiles_last = [t for t in range(NTT) if rsegs[t] == E - 1] if last_pin else []

    # ---- shared weight packing (global segment order) ----
    w1p = np.transpose(
        np.asarray(fc1_w).reshape(E, HC, P, DC, P), (0, 4, 1, 3, 2)
    ).reshape(E, P, W1C)
    w2p = np.transpose(
        np.asarray(fc2_w).reshape(E, D, HC, P), (0, 3, 2, 1)
    ).reshape(E, P, W2C)
    w_all = np.ascontiguousarray(
        np.concatenate([w1p, w2p], axis=2)[order]
    ).astype(bf)

    wg_r = np.asarray(w_gate, np.float32).reshape(FC, P, E)
    wn_r = np.asarray(w_noise, np.float32).reshape(FC, P, E)
    wgwn = np.ascontiguousarray(
        np.transpose(np.concatenate([wg_r, wn_r], axis=2), (1, 0, 2)).reshape(
            P, FC * 2 * E
        )
    ).astype(bf)

    use_b1 = bool(np.any(np.asarray(fc1_b)))
    use_b2 = bool(np.any(np.asarray(fc2_b)))
    b1p = np.ascontiguousarray(
        np.transpose(np.asarray(fc1_b, np.float32)[order].reshape(E, HC, P),
                     (2, 0, 1)).reshape(P, E * HC)
    )
    b2p = np.ascontiguousarray(np.asarray(fc2_b)[order]).astype(bf)

    # ---- pass 2: per-core input packing ----
    in_maps = []
    gidxs = []
    for c in range(NC):
        (pair_seg, sort_idx, seg_counts, seg_starts,
         rows_of_pair, ready, perm) = percore[c]
        j1 = rows_of_pair[0::2].astype(np.int32)
        j2 = rows_of_pair[1::2].astype(np.int32)

        # x columns in segment order, padded per segment (unpadded offsets)
        tok_sorted = sort_idx // 2
        cols = np.zeros(R, dtype=np.int64)
        for k in range(E):
            s0, n = int(seg_starts[k]), int(seg_counts[k])
            cols[offs[k] : offs[k] + n] = tok_sorted[s0 : s0 + n]
        if last_pin:
            cols[offs[E - 1] : offs[E - 1] + 2 * P] = perm[NS - 2 * P :]
        x_loc = x[toks[c]]
        xt = np.ascontiguousarray(
            np.transpose(x_loc[cols].reshape(R, DC, P), (2, 1, 0)).reshape(
                P, DC * R
            )
        ).astype(bf)

        gf = gate_feat[toks[c]][perm]
        gft = np.ascontiguousarray(
            np.transpose(gf.reshape(NS, FC, P), (2, 1, 0)).reshape(P, FC * NS)
        ).astype(bf)
        nst = np.ascontiguousarray(noise[toks[c]][perm].T.astype(np.float32))

        # j12 pairs adjacent: col 2t = top-1 row, 2t+1 = top-2 row of tile t.
        # tiles handled by combine_last get (jA = non-last-seg row, jB2 =
        # straggler fallback row) plus the s/m selectors in sm
        j1t = j1[perm].reshape(NTT, P)
        j2t = j2[perm].reshape(NTT, P)
        s71 = (pair_seg[0::2] == E - 1)[perm].reshape(NTT, P)
        s72 = (pair_seg[1::2] == E - 1)[perm].reshape(NTT, P)
        sm = np.zeros((P, 8), dtype=np.float32)
        for i, t in enumerate(tiles_last):
            has7 = s71[t] | s72[t]
            if t != tiles_last[0]:
                assert has7.all(), "stragglers must sit in the first last-tile"
            ja = np.where(s71[t], j2t[t], j1t[t])
            jb2 = np.where(has7, 0, j2t[t])
            j1t[t], j2t[t] = ja, jb2
            tt0 = t - (NTT - int(ntts[E - 1]))
            sm[:, 4 * tt0 + 0] = s71[t].astype(np.float32)
            sm[:, 4 * tt0 + 1] = has7.astype(np.float32)
        j12 = np.ascontiguousarray(
            np.stack([j1t.T, j2t.T], axis=2).reshape(P, 2 * NTT)
        )
        im = {
            "xt": xt,
            "gft": gft,
            "nst": nst,
            "wgwn": wgwn,
            "w": w_all,
            "j12": j12,
            "sm": sm,
        }
        if use_b1:
            im["b1"] = b1p
        if use_b2:
            im["b2"] = b2p
        in_maps.append(im)
        gidxs.append(toks[c][perm])

    return tuple(int(v) for v in caps), rsegs, gidxs, in_maps


def kernel(x, gate_feat, noise, w_gate, w_noise, fc1_w, fc1_b, fc2_w, fc2_b,
           _reps=1):
    caps, rsegs, gidxs, in_maps = _prepare(
        x, gate_feat, noise, w_gate, w_noise, fc1_w, fc1_b, fc2_w, fc2_b
    )
    use_b1 = "b1" in in_maps[0]
    use_b2 = "b2" in in_maps[0]
    key = (caps, rsegs, int(_reps), use_b1, use_b2)
    if key not in _nc_cache:
        _nc_cache[key] = _build_nc(caps, rsegs, reps=_reps, use_b1=use_b1,
                                   use_b2=use_b2)
    nc = _nc_cache[key]
    try:
        res = run_bass_kernel_spmd(nc, in_maps, core_ids=list(range(NC)))
    except Exception:
        # transient device wedge (seen once as NRT_EXEC_UNIT_UNRECOVERABLE on a
        # cold device); one retry after the runtime recovers
        res = run_bass_kernel_spmd(nc, in_maps, core_ids=list(range(NC)))
    y = np.empty((N, D), np.float32)
    for c in range(NC):
        y[gidxs[c]] = res.results[c]["y"]
    return y
